# revision 1
# baseline (speedup 1.0000x reference)
"""DOSACon loss on 8 Trainium2 NeuronCores (Bass/Tile, SPMD data-parallel).

Math: the [N,N] broadcast in the localization term is rank-1 separable --
  mean(dw * hw * (1-ciou)^g / (area+eps)) over [N,N]
    = (sum_i dw_i*hw_i*(1-ciou_i)^g) * (sum_j 1/(area_j+eps)) / N^2
so each core computes partial sums over its 1024-row shard of the N=8192
boxes.  The 100-pair contrastive term is gathered on host (pure data
movement) and evaluated on-device in a packed 9th column / pair lane.

Device layout per core: one packed [128, 592] f32 input buffer
  cols   0: 36  P2  = px|py|tx|ty   (4 blocks of 9: 8 shard cols + 1 pair col)
  cols  36: 72  WH  = pw|ph|tw|th   (same block structure)
  cols  72: 80  density shard
  cols  80:336  gathered embeddings i (rows 100.. zero padded)
  cols 336:592  gathered embeddings j
Output per core: [128, 3] partials (col0 = loc numerator terms, col1 =
inverse-area terms, col2 = per-pair masked hinge^2; rows 100.. of col2 are
pad garbage and ignored on host).

Engine plan: DVE carries the serial CIoU chain; Pool (gpsimd) takes the
independent sub-chains (enclosing-box square, center distance, embedding
diff); ACT ops are grouped so only two activation tables load (Arctan+Sigmoid
share "sigmoid_and_others", loaded during the preamble; the late Sqrt
loads "sqrt_and_others" once).
"""

from contextlib import ExitStack

import numpy as np

N_CORES = 8
N = 8192
NS = N // N_CORES      # 1024 boxes per core
PPART = 128            # SBUF partitions
FREE = NS // PPART     # 8 shard columns
W = FREE + 1           # 9 = shard columns + 1 pair column
D = 256
NPAIR = 100

GAMMA = 2.5
ALPHA_D = 1.2
DELTA = 1.0
TAU = 0.3
LAMBDA_C = 0.5
EPS = 1e-7

_BUILT = None          # cached nc across calls
LAST_RESULT = None     # last BassKernelResults (for profiling in test.py)


def _build_nc():
    import concourse.bacc as bacc
    import concourse.mybir as mybir
    import concourse.tile as tile
    from concourse.tile import add_dep_helper

    dt = mybir.dt.float32
    A = mybir.AluOpType
    AF = mybir.ActivationFunctionType
    AX = mybir.AxisListType

    nc = bacc.Bacc("TRN2", target_bir_lowering=False, debug=False,
                   num_devices=N_CORES)
    buf_d = nc.dram_tensor("buf", [PPART, 592], dt, kind="ExternalInput")
    out_d = nc.dram_tensor("out", [PPART, 3], dt, kind="ExternalOutput")

    with tile.TileContext(nc) as tc, ExitStack() as ctx:
        pool = ctx.enter_context(tc.tile_pool(name="p", bufs=1))

        def T(n, tag):
            return pool.tile([PPART, n], dt, name=tag, tag=tag)

        # two tiles so box math only waits on the small first DMA, not on
        # the embeddings transfer (Tile tracks deps per tile)
        bufA = T(80, "bufA")
        bufB = T(512, "bufB")
        nc.sync.dma_start(bufA[:], buf_d.ap()[:, 0:80])
        nc.sync.dma_start(bufB[:], buf_d.ap()[:, 80:592])

        P2 = bufA[:, 0:36]      # px|py|tx|ty
        WH = bufA[:, 36:72]     # pw|ph|tw|th
        dn = bufA[:, 72:80]
        ei = bufB[:, 0:256]
        ej = bufB[:, 256:512]
        whr = WH.rearrange("p (a b) -> p a b", b=W)
        w_in = whr[:, 0::2, :]  # pw|tw  [128,2,9]
        h_in = whr[:, 1::2, :]  # ph|th  [128,2,9]

        def r2(ap):             # view a [128,18] tile as [128,2,9]
            return ap.rearrange("p (a b) -> p a b", b=W)

        V, S, G = nc.vector, nc.scalar, nc.gpsimd

        # === DVE: aspect-ratio chain first so Arctan is the first ACT op
        # (its table then loads during the preamble; Sigmoid shares it and
        # only the late Sqrt needs a second table load).
        # HW Arctan only covers [-pi/2, pi/2]; ratios are in (0, inf), so
        # use arctan(x) = pi/4 + arctan((x-1)/(x+1)) -- the pi/4 shift
        # cancels in the arctan difference, and (x-1)/(x+1) is in (-1, 1).
        rh = T(18, "rh")
        V.reciprocal(r2(rh[:]), h_in)
        rat = T(18, "rat")
        V.tensor_tensor(r2(rat[:]), w_in, r2(rh[:]), A.mult)
        zd = T(18, "zd")
        V.tensor_scalar_add(zd[:], rat[:], 1.0)
        rzd = T(18, "rzd")
        V.reciprocal(rzd[:], zd[:])
        z = T(18, "z")      # (x-1)/(x+1) = 1 - 2/(x+1); x=inf -> z=1, no NaN
        V.tensor_scalar(z[:], rzd[:], -2.0, 1.0, A.mult, A.add)
        ats = T(18, "ats")
        S.activation(ats[:], z[:], AF.Arctan)

        # === Pool: independent work first (in-order engine; no stalls)
        b25 = T(1, "b25")
        G.memset(b25[:], 2.5)
        dwt = T(FREE, "dwt")    # 1 + 1.2*density
        G.tensor_scalar(dwt[:], dn, ALPHA_D, 1.0, A.mult, A.add)
        ar = T(18, "ar")        # a1|a2 = pw*ph | tw*th
        G.tensor_tensor(r2(ar[:]), w_in, h_in, A.mult)
        u0 = T(W, "u0")
        G.tensor_tensor(u0[:], ar[:, 0:W], ar[:, W:2 * W], A.add)
        u0e = T(W, "u0e")       # a1 + a2 + EPS
        G.tensor_scalar(u0e[:], u0[:], EPS, None, A.add)
        ad = T(FREE, "ad")
        G.tensor_scalar(ad[:], ar[:, W:W + FREE], 1e-7, None, A.add)
        dxy = T(18, "dxy")
        G.tensor_tensor(dxy[:], P2[:, 18:36], P2[:, 0:18], A.subtract)
        dsq = T(18, "dsq")
        G.tensor_tensor(dsq[:], dxy[:], dxy[:], A.mult)
        rho2 = T(W, "rho2")
        G.tensor_tensor(rho2[:], dsq[:, 0:W], dsq[:, W:2 * W], A.add)
        diff = T(D, "diff")
        diff_inst = G.tensor_tensor(diff[:], ei, ej, A.subtract)
        sq2 = T(D, "sq2")
        G.tensor_tensor(sq2[:], diff[:], diff[:], A.mult)

        # === DVE: corners / intersection / union / iou
        lo = T(36, "lo")        # b1x1|b1y1|b2x1|b2y1
        hi = T(36, "hi")        # b1x2|b1y2|b2x2|b2y2
        V.scalar_tensor_tensor(lo[:], WH, -0.5, P2, A.mult, A.add)
        V.scalar_tensor_tensor(hi[:], WH, 0.5, P2, A.mult, A.add)
        mlo = T(18, "mlo")
        mhi = T(18, "mhi")
        V.tensor_tensor(mlo[:], lo[:, 0:18], lo[:, 18:36], A.max)
        V.tensor_tensor(mhi[:], hi[:, 0:18], hi[:, 18:36], A.min)
        iwh = T(18, "iwh")      # unclipped intersection extents
        V.tensor_tensor(iwh[:], mhi[:], mlo[:], A.subtract)
        iwr = T(18, "iwr")
        V.tensor_scalar_max(iwr[:], iwh[:], 0.0)
        inter = T(W, "inter")
        V.tensor_tensor(inter[:], iwr[:, 0:W], iwr[:, W:2 * W], A.mult)
        u2 = T(W, "u2")         # union = u0e - inter
        V.scalar_tensor_tensor(u2[:], inter[:], -1.0, u0e[:], A.mult, A.add)
        ru = T(W, "ru")
        V.reciprocal(ru[:], u2[:])
        iou = T(W, "iou")
        V.tensor_tensor(iou[:], inter[:], ru[:], A.mult)
        c0 = T(18, "c0")
        c1 = T(18, "c1")
        V.tensor_tensor(c0[:], hi[:, 0:18], hi[:, 18:36], A.max)
        V.tensor_tensor(c1[:], lo[:, 0:18], lo[:, 18:36], A.min)
        stats = T(3, "stats")

        # === Pool: enclosing-box chain (c0/c1 handed off from DVE)
        cwh = T(18, "cwh")
        G.tensor_tensor(cwh[:], c0[:], c1[:], A.subtract)
        csq = T(18, "csq")
        G.tensor_tensor(csq[:], cwh[:], cwh[:], A.mult)
        c2 = T(W, "c2")
        c2_inst = G.tensor_tensor(c2[:], csq[:, 0:W], csq[:, W:2 * W], A.add)
        # Pool runs in order: embeddings diff must not delay the c-chain
        add_dep_helper(diff_inst.ins, c2_inst.ins, sync=False,
                       reason="order Pool diff after c2")
        mask = T(1, "mask")
        G.tensor_scalar(mask[:], iou[:, FREE:W], TAU, None, A.is_gt)

        # === DVE: alpha chain; v = VS*dv2 with VS = 4/pi^2 folded in
        VS = 4.0 / np.pi ** 2
        dv = T(W, "dv")
        V.tensor_tensor(dv[:], ats[:, W:2 * W], ats[:, 0:W], A.subtract)
        dv2 = T(W, "dv2")
        V.tensor_tensor(dv2[:], dv[:], dv[:], A.mult)
        d0 = T(W, "d0")         # v - iou
        V.scalar_tensor_tensor(d0[:], dv2[:], VS, iou[:], A.mult, A.subtract)
        d1 = T(W, "d1")
        V.tensor_scalar_add(d1[:], d0[:], 1.0 + EPS)
        rd = T(W, "rd")
        V.reciprocal(rd[:], d1[:])
        vv = T(W, "vv")         # dv2^2
        V.tensor_tensor(vv[:], dv2[:], dv2[:], A.mult)
        va = T(W, "va")         # v^2/d1 = v*alpha
        va_inst = V.scalar_tensor_tensor(va[:], vv[:], VS * VS, rd[:],
                                         A.mult, A.mult)
        c2e = T(W, "c2e")
        c2e_inst = V.tensor_scalar_add(c2e[:], c2[:], EPS)
        # keep the v/alpha chain ahead of the c-chain tail on DVE
        add_dep_helper(c2e_inst.ins, va_inst.ins, sync=False,
                       reason="order c2e after va on DVE")
        rc2 = T(W, "rc2")
        V.reciprocal(rc2[:], c2e[:])
        rr = T(W, "rr")         # rho2 / c2
        V.tensor_tensor(rr[:], rho2[:], rc2[:], A.mult)
        pen = T(W, "pen")
        V.tensor_tensor(pen[:], rr[:], va[:], A.add)
        ciou = T(W, "ciou")
        V.tensor_tensor(ciou[:], iou[:], pen[:], A.subtract)

        omd = T(W, "omd")       # cols 0:8 = 1-ciou, col 8 = |ei-ej|^2
        om_inst = V.tensor_scalar(omd[:, 0:FREE], ciou[:, 0:FREE], -1.0, 1.0,
                                  A.mult, A.add)
        d2_inst = V.tensor_reduce(omd[:, FREE:W], sq2[:], axis=AX.X, op=A.add)
        # keep the d2 reduce out of the alpha chain's way on DVE: without
        # this the scheduler hoists it and DVE stalls on Pool's sq2
        add_dep_helper(d2_inst.ins, om_inst.ins, sync=False,
                       reason="order d2-reduce after om on DVE")

        # === ACT tail: Sigmoid first (needs only ciou, so the sqrt-table
        # load right after it starts ~0.5us earlier than the reverse order;
        # the load overlaps the DVE hin/p25 tail work).
        hwt = T(FREE, "hwt")    # sigmoid(5*(0.5-ciou))
        sig_inst = S.activation(hwt[:], ciou[:, 0:FREE], AF.Sigmoid,
                                scale=-5.0, bias=b25[:])
        st = T(W, "st")         # sqrt(om) | dist
        sqrt_inst = S.activation(st[:], omd[:], AF.Sqrt)
        add_dep_helper(sqrt_inst.ins, sig_inst.ins, sync=False,
                       reason="sigmoid before sqrt on ACT")

        # === Pool tail (sq/p25 only need om/st; m1/scr wait for sigmoid)
        sq = T(FREE, "sq")
        G.tensor_tensor(sq[:], omd[:, 0:FREE], omd[:, 0:FREE], A.mult)
        p25 = T(FREE, "p25")    # (1-ciou)^2.5
        G.tensor_tensor(p25[:], sq[:], st[:, 0:FREE], A.mult)

        # === DVE tail (ia/reduce1 fill the table-load idle windows)
        ia = T(FREE, "ia")
        ia_inst = V.reciprocal(ia[:], ad[:])
        add_dep_helper(ia_inst.ins, om_inst.ins, sync=False,
                       reason="keep ia out of the pre-om DVE stream")
        V.tensor_reduce(stats[:, 1:2], ia[:], axis=AX.X, op=A.add)
        hin = T(1, "hin")       # relu(DELTA - dist)
        V.tensor_scalar(hin[:], st[:, FREE:W], -1.0, DELTA, A.mult, A.add)
        hinr = T(1, "hinr")
        V.tensor_scalar_max(hinr[:], hin[:], 0.0)
        h2 = T(1, "h2")
        V.tensor_tensor(h2[:], hinr[:], hinr[:], A.mult)
        m1 = T(FREE, "m1")
        V.tensor_tensor(m1[:], dwt[:], hwt[:], A.mult)
        scr = T(FREE, "scr")
        V.tensor_tensor(scr[:], m1[:], p25[:], A.mult)
        V.tensor_reduce(stats[:, 0:1], scr[:], axis=AX.X, op=A.add)
        V.tensor_tensor(stats[:, 2:3], mask[:], h2[:], A.mult)

        nc.sync.dma_start(out_d.ap(), stats[:])

    nc.compile()
    return nc


def _get_nc():
    global _BUILT
    if _BUILT is None:
        _BUILT = _build_nc()
    return _BUILT


def _pack_inputs(pred_boxes, target_boxes, embeddings, density_map, indices):
    pred = np.ascontiguousarray(pred_boxes, dtype=np.float32)
    targ = np.ascontiguousarray(target_boxes, dtype=np.float32)
    emb = np.ascontiguousarray(embeddings, dtype=np.float32)
    dens = np.ascontiguousarray(density_map, dtype=np.float32)
    idx = np.asarray(indices).astype(np.int64)

    i0, i1 = idx[:, 0], idx[:, 1]
    bi = np.ones((PPART, 4), np.float32)
    bj = np.ones((PPART, 4), np.float32)
    bi[:NPAIR] = pred[i0]
    bj[:NPAIR] = pred[i1]
    ei = np.zeros((PPART, D), np.float32)
    ej = np.zeros((PPART, D), np.float32)
    ei[:NPAIR] = emb[i0]
    ej[:NPAIR] = emb[i1]

    in_maps = []
    for c in range(N_CORES):
        s = slice(c * NS, (c + 1) * NS)
        pbs = pred[s].reshape(PPART, FREE, 4)
        tbs = targ[s].reshape(PPART, FREE, 4)
        buf = np.empty((PPART, 592), np.float32)
        # P2 blocks: px py tx ty ; WH blocks: pw ph tw th
        for k, (src, comp) in enumerate(
                [(pbs, 0), (pbs, 1), (tbs, 0), (tbs, 1),
                 (pbs, 2), (pbs, 3), (tbs, 2), (tbs, 3)]):
            pair = (bi if src is pbs else bj)[:, comp]
            buf[:, k * W:k * W + FREE] = src[:, :, comp]
            buf[:, k * W + FREE] = pair
        buf[:, 72:80] = dens[s].reshape(PPART, FREE)
        buf[:, 80:336] = ei
        buf[:, 336:592] = ej
        in_maps.append({"buf": buf})
    return in_maps


def kernel(pred_boxes, target_boxes, embeddings, density_map, indices):
    global LAST_RESULT
    import time as _time

    from concourse.bass_utils import run_bass_kernel_spmd

    nc = _get_nc()
    in_maps = _pack_inputs(pred_boxes, target_boxes, embeddings,
                           density_map, indices)
    for attempt in range(3):
        try:
            res = run_bass_kernel_spmd(nc, in_maps,
                                       core_ids=list(range(N_CORES)))
            break
        except Exception:
            # a crashed earlier run can leave a core wedged
            # (NRT_EXEC_UNIT_UNRECOVERABLE); it clears on retry
            if attempt == 2:
                raise
            _time.sleep(2.0)
    LAST_RESULT = res

    stats = np.stack([res.results[c]["out"] for c in range(N_CORES)])
    s_a = float(np.sum(stats[:, :, 0], dtype=np.float64))
    s_b = float(np.sum(stats[:, :, 1], dtype=np.float64))
    contrast = float(np.sum(stats[0, :NPAIR, 2], dtype=np.float64))
    loss = s_a * s_b / (N * N) + LAMBDA_C * contrast / (NPAIR + 1e-7)
    return np.asarray(np.float32(loss))



# revision 12
# speedup vs baseline: 1.0741x; 1.0741x over previous
"""DOSACon loss on 8 Trainium2 NeuronCores (Bass/Tile, SPMD data-parallel).

Math: the [N,N] broadcast in the localization term is rank-1 separable --
  mean(dw * hw * (1-ciou)^g / (area+eps)) over [N,N]
    = (sum_i dw_i*hw_i*(1-ciou_i)^g) * (sum_j 1/(area_j+eps)) / N^2
so each core computes partial sums over its 1024-row shard of the N=8192
boxes.  The 100 contrastive pairs are gathered on host (pure data
movement) and SHARDED across cores (13 per core) in a packed pair lane.

Single activation table (#6: ln/exp):
  sigmoid(5(0.5-ciou)) = 1/(1+exp(-5*om+2.5))   with om = 1-ciou
  (1-ciou)^2.5        = exp(2.5*ln(om))
  ||ei-ej||           = exp(0.5*ln(d2+1e-12))
  arctan              = odd deg-5 polynomial of z=(w-h)/(w+h) (1.4e-5
                        end-to-end error), so no Arctan table needed.

Per-core inputs: bufA [128, 80] f32 (box data: P2|WH|density, 9-wide
blocks = 8 shard cols + 1 pair col), bufB [13, 512] (13 pairs ei|ej).
Output: [1, 3] = (sum_a, sum_b, sum_pair) after an on-device partition
reduction via PE matmul against a ones column -- a single 12B DMA packet.

Engine plan: DVE owns the serial CIoU chain; Pool does the arctan poly,
area/density prep, embedding diff, mask; ACT does the ln/exp ops plus the
squared-distance accumulation (Square with accum_out); PE does the final
partition reduce.  Fused reduce: scr/ib use accum_out to fold the X-axis
reduction into the last elementwise op.
"""

from contextlib import ExitStack

import numpy as np

N_CORES = 8
N = 8192
NS = N // N_CORES      # 1024 boxes per core
PPART = 128            # SBUF partitions
FREE = NS // PPART     # 8 shard columns
W = FREE + 1           # 9 = shard columns + 1 pair column
D = 256
NPAIR = 100
PAIRS_PER = 13         # ceil(100/8); tail cores padded with dummies

GAMMA = 2.5
ALPHA_D = 1.2
DELTA = 1.0
TAU = 0.3
LAMBDA_C = 0.5
EPS = 1e-7
VS = 4.0 / np.pi ** 2
# odd minimax-ish arctan poly on [-1,1]: c1*z + c3*z^3 + c5*z^5
AT_C1 = 0.99570612
AT_C3 = -0.29065729
AT_C5 = 0.08132208

_BUILT = None          # cached nc across calls
LAST_RESULT = None     # last BassKernelResults (for profiling in test.py)


def _build_nc():
    import concourse.bacc as bacc
    import concourse.mybir as mybir
    import concourse.tile as tile

    dt = mybir.dt.float32
    A = mybir.AluOpType
    AF = mybir.ActivationFunctionType

    nc = bacc.Bacc("TRN2", target_bir_lowering=False, debug=False,
                   num_devices=N_CORES)
    bufA_d = nc.dram_tensor("bufA", [PPART, 80], dt, kind="ExternalInput")
    bufB_d = nc.dram_tensor("bufB", [PAIRS_PER, 2 * D], dt,
                            kind="ExternalInput")
    out_d = nc.dram_tensor("out", [1, 3], dt, kind="ExternalOutput")

    with tile.TileContext(nc) as tc, ExitStack() as ctx:
        pool = ctx.enter_context(tc.tile_pool(name="p", bufs=1))
        ppool = ctx.enter_context(tc.psum_pool(name="pp", bufs=1))

        def T(n, tag, p=PPART):
            return pool.tile([p, n], dt, name=tag, tag=tag)

        V, S, G = nc.vector, nc.scalar, nc.gpsimd

        bufA = T(80, "bufA")
        bufB = T(2 * D, "bufB")
        nc.sync.dma_start(bufA[:], bufA_d.ap())
        S.dma_start(bufB[0:PAIRS_PER, :], bufB_d.ap())

        P2 = bufA[:, 0:36]      # px|py|tx|ty (9-wide blocks)
        WH = bufA[:, 36:72]     # pw|ph|tw|th
        dn = bufA[:, 72:80]
        whr = WH.rearrange("p (a b) -> p a b", b=W)
        w_in = whr[:, 0::2, :]  # pw|tw  [128,2,9]
        h_in = whr[:, 1::2, :]  # ph|th  [128,2,9]

        def r2(ap):             # view a [128,18] tile as [128,2,9]
            return ap.rearrange("p (a b) -> p a b", b=W)

        # === Pool preamble (no data deps: runs during the input DMA) ===
        stats = T(3, "stats")
        ones8 = T(FREE, "ones8")
        b25 = T(1, "b25")
        beps = T(1, "beps")
        G.memset(stats[:, 2:3], 0.0)
        G.memset(ones8[:], 1.0)
        G.memset(b25[:], 2.5)
        G.memset(beps[:], 1e-12)

        # === Pool: embedding diff first (bufB lands before bufA) ===
        diff = T(D, "diff")
        G.tensor_tensor(diff[:], bufB[:, 0:D], bufB[:, D:2 * D], A.subtract)
        # ACT: d2 = sum((ei-ej)^2) fused square+row-reduce
        sq2 = T(D, "sq2")
        d2c = T(1, "d2c")
        S.activation(sq2[:], diff[:], AF.Square, accum_out=d2c[:])
        lnd2 = T(1, "lnd2")
        S.activation(lnd2[:], d2c[:], AF.Ln, bias=beps[:])
        dist = T(1, "dist")
        S.activation(dist[:], lnd2[:], AF.Exp, scale=0.5)
        rlu = T(1, "rlu")       # relu(1 - dist)
        S.activation(rlu[:], dist[:], AF.Relu, scale=-1.0, bias=1.0)

        # === Pool: box-side prep ===
        za = T(18, "za")        # w - h (pred | targ)
        zb = T(18, "zb")        # w + h
        G.tensor_tensor(r2(za[:]), w_in, h_in, A.subtract)
        G.tensor_tensor(r2(zb[:]), w_in, h_in, A.add)
        dwt = T(FREE, "dwt")    # 1 + 1.2*density
        G.tensor_scalar(dwt[:], dn, ALPHA_D, 1.0, A.mult, A.add)
        ar = T(18, "ar")        # a1|a2 = pw*ph | tw*th
        G.tensor_tensor(r2(ar[:]), w_in, h_in, A.mult)
        u0 = T(W, "u0")
        G.tensor_tensor(u0[:], ar[:, 0:W], ar[:, W:2 * W], A.add)
        u0e = T(W, "u0e")       # a1 + a2 + EPS
        G.tensor_scalar(u0e[:], u0[:], EPS, None, A.add)
        ad = T(FREE, "ad")      # area + 1e-7 (shard cols of tw*th)
        G.tensor_scalar(ad[:], ar[:, W:W + FREE], 1e-7, None, A.add)

        # === DVE: arctan ratio + corners chain ===
        rzb = T(18, "rzb")
        V.reciprocal(rzb[:], zb[:])
        z = T(18, "z")
        V.tensor_tensor(z[:], za[:], rzb[:], A.mult)
        uu = T(18, "uu")        # z^2
        V.tensor_tensor(uu[:], z[:], z[:], A.mult)
        lo = T(36, "lo")        # b1x1|b1y1|b2x1|b2y1
        hi = T(36, "hi")        # b1x2|b1y2|b2x2|b2y2
        V.scalar_tensor_tensor(lo[:], WH, -0.5, P2, A.mult, A.add)
        V.scalar_tensor_tensor(hi[:], WH, 0.5, P2, A.mult, A.add)
        mlo = T(18, "mlo")
        mhi = T(18, "mhi")
        V.tensor_tensor(mlo[:], lo[:, 0:18], lo[:, 18:36], A.max)
        V.tensor_tensor(mhi[:], hi[:, 0:18], hi[:, 18:36], A.min)
        iwh = T(18, "iwh")
        V.tensor_tensor(iwh[:], mhi[:], mlo[:], A.subtract)
        iwr = T(18, "iwr")
        V.tensor_scalar_max(iwr[:], iwh[:], 0.0)
        inter = T(W, "inter")
        V.tensor_tensor(inter[:], iwr[:, 0:W], iwr[:, W:2 * W], A.mult)
        u2 = T(W, "u2")         # union = u0e - inter
        V.scalar_tensor_tensor(u2[:], inter[:], -1.0, u0e[:], A.mult, A.add)
        ru = T(W, "ru")
        V.reciprocal(ru[:], u2[:])
        iou = T(W, "iou")
        V.tensor_tensor(iou[:], inter[:], ru[:], A.mult)
        w1p = T(W, "w1p")       # (1+EPS) - iou
        V.tensor_scalar(w1p[:], iou[:], -1.0, 1.0 + EPS, A.mult, A.add)

        # === Pool: arctan poly head (needs uu from DVE) ===
        h1 = T(18, "h1")        # c3 + c5*u
        G.tensor_scalar(h1[:], uu[:], AT_C5, AT_C3, A.mult, A.add)
        hu = T(18, "hu")
        G.tensor_tensor(hu[:], h1[:], uu[:], A.mult)
        # DVE: ats = (hu + c1) * z   (STT only exists on DVE)
        ats = T(18, "ats")
        V.scalar_tensor_tensor(ats[:], hu[:], AT_C1, z[:], A.add, A.mult)
        # Pool: v pieces
        dv = T(W, "dv")
        G.tensor_tensor(dv[:], ats[:, W:2 * W], ats[:, 0:W], A.subtract)
        dv2 = T(W, "dv2")
        G.tensor_tensor(dv2[:], dv[:], dv[:], A.mult)
        vv = T(W, "vv")         # dv2^2
        G.tensor_tensor(vv[:], dv2[:], dv2[:], A.mult)

        # === DVE: rho2 / enclosing-box chains (fill the dv2 wait) ===
        c0 = T(18, "c0")
        c1 = T(18, "c1")
        V.tensor_tensor(c0[:], hi[:, 0:18], hi[:, 18:36], A.max)
        V.tensor_tensor(c1[:], lo[:, 0:18], lo[:, 18:36], A.min)
        dxy = T(18, "dxy")
        V.tensor_tensor(dxy[:], P2[:, 18:36], P2[:, 0:18], A.subtract)
        dsq = T(18, "dsq")
        V.tensor_tensor(dsq[:], dxy[:], dxy[:], A.mult)
        rho2 = T(W, "rho2")
        V.tensor_tensor(rho2[:], dsq[:, 0:W], dsq[:, W:2 * W], A.add)
        cwh = T(18, "cwh")
        V.tensor_tensor(cwh[:], c0[:], c1[:], A.subtract)
        csq = T(18, "csq")
        V.tensor_tensor(csq[:], cwh[:], cwh[:], A.mult)
        c2e = T(W, "c2e")       # cw^2 + ch^2 + EPS
        V.scalar_tensor_tensor(c2e[:], csq[:, 0:W], EPS, csq[:, W:2 * W],
                               A.add, A.add)
        rc2 = T(W, "rc2")
        V.reciprocal(rc2[:], c2e[:])
        rr = T(W, "rr")         # rho2 / c2
        V.tensor_tensor(rr[:], rho2[:], rc2[:], A.mult)
        omirr = T(W, "omirr")   # (1-iou) + rho2/c2   (+EPS, negligible)
        V.tensor_tensor(omirr[:], w1p[:], rr[:], A.add)
        d1 = T(W, "d1")         # v + (1+EPS) - iou
        V.scalar_tensor_tensor(d1[:], dv2[:], VS, w1p[:], A.mult, A.add)
        rd = T(W, "rd")
        V.reciprocal(rd[:], d1[:])
        va = T(W, "va")         # v^2 / d1 = (VS^2*vv) * rd = v*alpha
        V.scalar_tensor_tensor(va[:], vv[:], VS * VS, rd[:],
                               A.mult, A.mult)
        om = T(W, "om")         # 1 - ciou
        V.tensor_tensor(om[:], omirr[:], va[:], A.add)

        # === Pool: mask (pair col of iou) + h2 ===
        mask = pool.tile([PPART, 1], mybir.dt.int32, name="mask", tag="mask")
        G.tensor_scalar(mask[:], iou[:, FREE:W], TAU, None, A.is_gt)
        h2 = T(1, "h2")
        G.tensor_tensor(h2[:], rlu[:], rlu[:], A.mult)

        # === ACT tail: e5 | ln(om) | p25 ===
        e5 = T(FREE, "e5")      # exp(-5*om + 2.5)
        S.activation(e5[:], om[:, 0:FREE], AF.Exp, scale=-5.0, bias=b25[:])
        lnom = T(FREE, "lnom")
        S.activation(lnom[:], om[:, 0:FREE], AF.Ln)
        p25 = T(FREE, "p25")    # om^2.5
        S.activation(p25[:], lnom[:], AF.Exp, scale=GAMMA)

        # === DVE tail ===
        # b-partial: 1/ad with fused row-reduce into stats col 1
        ia = T(FREE, "ia")
        V.reciprocal(ia[:], ad[:])
        ib = T(FREE, "ib")
        V.tensor_scalar(ib[:], ia[:], 1.0, 0.0, A.mult, A.add,
                        accum_out=stats[:, 1:2])
        t1 = T(FREE, "t1")      # 1 + e5
        V.tensor_scalar_add(t1[:], e5[:], 1.0)
        rt = T(FREE, "rt")
        V.reciprocal(rt[:], t1[:])
        m1 = T(FREE, "m1")      # dw * hw = dwt * rt
        V.tensor_tensor(m1[:], dwt[:], rt[:], A.mult)
        scr = T(FREE, "scr")    # m1 * p25, row-reduced into stats col 0
        V.scalar_tensor_tensor(scr[:], m1[:], 1.0, p25[:], A.mult, A.mult,
                               accum_out=stats[:, 0:1])
        V.copy_predicated(stats[:, 2:3], mask[:], h2[:])

        # === PE: partition reduce -> [1,3]; single-packet DMA out ===
        pt = ppool.tile([PPART, 3], dt, name="pt", tag="pt")
        nc.tensor.matmul(pt[0:1, :], ones8[:, 0:1], stats[:],
                         start=True, stop=True)
        outs = pool.tile([1, 3], dt, name="outs", tag="outs")
        S.activation(outs[:], pt[0:1, :], AF.Copy)
        nc.sync.dma_start(out_d.ap(), outs[:])

    nc.compile()
    return nc


def _get_nc():
    global _BUILT
    if _BUILT is None:
        _BUILT = _build_nc()
    return _BUILT


def _pack_inputs(pred_boxes, target_boxes, embeddings, density_map, indices):
    pred = np.ascontiguousarray(pred_boxes, dtype=np.float32)
    targ = np.ascontiguousarray(target_boxes, dtype=np.float32)
    emb = np.ascontiguousarray(embeddings, dtype=np.float32)
    dens = np.ascontiguousarray(density_map, dtype=np.float32)
    idx = np.asarray(indices).astype(np.int64)

    i0, i1 = idx[:, 0], idx[:, 1]
    # dummy pad pairs: far-apart unit boxes -> iou 0 -> mask 0
    bi_all = np.tile(np.array([1.0, 1.0, 1.0, 1.0], np.float32),
                     (N_CORES * PAIRS_PER, 1))
    bj_all = np.tile(np.array([9.0, 9.0, 1.0, 1.0], np.float32),
                     (N_CORES * PAIRS_PER, 1))
    ei_all = np.zeros((N_CORES * PAIRS_PER, D), np.float32)
    ej_all = np.zeros((N_CORES * PAIRS_PER, D), np.float32)
    bi_all[:NPAIR] = pred[i0]
    bj_all[:NPAIR] = pred[i1]
    ei_all[:NPAIR] = emb[i0]
    ej_all[:NPAIR] = emb[i1]

    in_maps = []
    for c in range(N_CORES):
        s = slice(c * NS, (c + 1) * NS)
        pbs = pred[s].reshape(PPART, FREE, 4)
        tbs = targ[s].reshape(PPART, FREE, 4)
        ps = slice(c * PAIRS_PER, (c + 1) * PAIRS_PER)
        bi, bj = bi_all[ps], bj_all[ps]
        bufA = np.empty((PPART, 80), np.float32)
        # P2 blocks: px py tx ty ; WH blocks: pw ph tw th
        for k, (src, comp) in enumerate(
                [(pbs, 0), (pbs, 1), (tbs, 0), (tbs, 1),
                 (pbs, 2), (pbs, 3), (tbs, 2), (tbs, 3)]):
            pair = (bi if src is pbs else bj)[:, comp]
            bufA[:, k * W:k * W + FREE] = src[:, :, comp]
            bufA[:PAIRS_PER, k * W + FREE] = pair
            bufA[PAIRS_PER:, k * W + FREE] = 1.0 if src is pbs else 9.0
            if comp < 2 and src is not pbs:
                pass
        # fix pad rows of pair col: w/h must be 1.0 for both
        for k, (src, comp) in enumerate(
                [(pbs, 0), (pbs, 1), (tbs, 0), (tbs, 1),
                 (pbs, 2), (pbs, 3), (tbs, 2), (tbs, 3)]):
            if comp >= 2:
                bufA[PAIRS_PER:, k * W + FREE] = 1.0
        bufA[:, 72:80] = dens[s].reshape(PPART, FREE)
        bufB = np.concatenate([ei_all[ps], ej_all[ps]], axis=1)
        in_maps.append({"bufA": bufA, "bufB": np.ascontiguousarray(bufB)})
    return in_maps


def kernel(pred_boxes, target_boxes, embeddings, density_map, indices):
    global LAST_RESULT
    import time as _time

    from concourse.bass_utils import run_bass_kernel_spmd

    nc = _get_nc()
    in_maps = _pack_inputs(pred_boxes, target_boxes, embeddings,
                           density_map, indices)
    for attempt in range(3):
        try:
            res = run_bass_kernel_spmd(nc, in_maps,
                                       core_ids=list(range(N_CORES)))
            break
        except Exception:
            # a crashed earlier run can leave a core wedged
            # (NRT_EXEC_UNIT_UNRECOVERABLE); it clears on retry
            if attempt == 2:
                raise
            _time.sleep(2.0)
    LAST_RESULT = res

    outs = np.stack([res.results[c]["out"] for c in range(N_CORES)])  # [8,1,3]
    s_a = float(np.sum(outs[:, 0, 0], dtype=np.float64))
    s_b = float(np.sum(outs[:, 0, 1], dtype=np.float64))
    contrast = float(np.sum(outs[:, 0, 2], dtype=np.float64))
    loss = s_a * s_b / (N * N) + LAMBDA_C * contrast / (NPAIR + 1e-7)
    return np.asarray(np.float32(loss))


# revision 14
# speedup vs baseline: 1.2298x; 1.1449x over previous
"""DOSACon loss on 8 Trainium2 NeuronCores (Bass/Tile, SPMD data-parallel).

Math: the [N,N] broadcast in the localization term is rank-1 separable --
  mean(dw * hw * (1-ciou)^g / (area+eps)) over [N,N]
    = (sum_i dw_i*hw_i*(1-ciou_i)^g) * (sum_j 1/(area_j+eps)) / N^2
so each core computes partial sums over its 1024-row shard of the N=8192
boxes.  The 100 contrastive pairs are gathered on host (pure data
movement) and SHARDED across cores (13 per core) in a packed pair lane.

Single activation table (#6: ln/exp):
  sigmoid(5(0.5-ciou)) = 1/(1+exp(-5*om+2.5))   with om = 1-ciou
  (1-ciou)^2.5        = exp(2.5*ln(om))
  ||ei-ej||           = exp(0.5*ln(d2+1e-12))
  arctan              = odd deg-5 polynomial of z=(w-h)/(w+h) (1.4e-5
                        end-to-end error), so no Arctan table needed.

Per-core inputs: bufA [128, 80] f32 (box data: P2|WH|density, 9-wide
blocks = 8 shard cols + 1 pair col), bufB [13, 512] (13 pairs ei|ej).
Output: [1, 3] = (sum_a, sum_b, sum_pair) after an on-device partition
reduction via PE matmul against a ones column -- a single 12B DMA packet.

Engine plan: DVE owns the serial CIoU chain; Pool does the arctan poly,
area/density prep, embedding diff, mask; ACT does the ln/exp ops plus the
squared-distance accumulation (Square with accum_out); PE does the final
partition reduce.  Fused reduce: scr/ib use accum_out to fold the X-axis
reduction into the last elementwise op.
"""

from contextlib import ExitStack

import numpy as np

N_CORES = 8
N = 8192
NS = N // N_CORES      # 1024 boxes per core
PPART = 128            # SBUF partitions
FREE = NS // PPART     # 8 shard columns
W = FREE + 1           # 9 = shard columns + 1 pair column
D = 256
NPAIR = 100
PAIRS_PER = 13         # ceil(100/8); tail cores padded with dummies

GAMMA = 2.5
ALPHA_D = 1.2
DELTA = 1.0
TAU = 0.3
LAMBDA_C = 0.5
EPS = 1e-7
VS = 4.0 / np.pi ** 2
# odd minimax-ish arctan poly on [-1,1]: c1*z + c3*z^3 + c5*z^5
AT_C1 = 0.99570612
AT_C3 = -0.29065729
AT_C5 = 0.08132208

_BUILT = None          # cached nc across calls
LAST_RESULT = None     # last BassKernelResults (for profiling in test.py)


def _build_nc():
    import concourse.bacc as bacc
    import concourse.mybir as mybir
    import concourse.tile as tile

    dt = mybir.dt.float32
    A = mybir.AluOpType
    AF = mybir.ActivationFunctionType

    nc = bacc.Bacc("TRN2", target_bir_lowering=False, debug=False,
                   num_devices=N_CORES)
    bufA_d = nc.dram_tensor("bufA", [PPART, 80], dt, kind="ExternalInput")
    bufB_d = nc.dram_tensor("bufB", [PAIRS_PER, 2 * D], dt,
                            kind="ExternalInput")
    out_d = nc.dram_tensor("out", [1, 3], dt, kind="ExternalOutput")

    with tile.TileContext(nc) as tc, ExitStack() as ctx:
        pool = ctx.enter_context(tc.tile_pool(name="p", bufs=1))
        ppool = ctx.enter_context(tc.psum_pool(name="pp", bufs=1))

        def T(n, tag, p=PPART):
            return pool.tile([p, n], dt, name=tag, tag=tag)

        V, S, G = nc.vector, nc.scalar, nc.gpsimd

        # load act table 6 (ln/exp/square/relu/copy) once, up front; the
        # compiler's per-function greedy table choice would thrash 0<->5.
        tl = mybir.InstLoadActFuncSet(
            name=nc.get_next_instruction_name(), ins=[], outs=[])
        tl.act_func_set_id = 6
        S.add_instruction(tl)

        bufA = T(80, "bufA")
        bufB = T(2 * D, "bufB")
        nc.sync.dma_start(bufA[:], bufA_d.ap())
        nc.sync.dma_start(bufB[0:PAIRS_PER, :], bufB_d.ap())

        P2 = bufA[:, 0:36]      # px|py|tx|ty (9-wide blocks)
        WH = bufA[:, 36:72]     # pw|ph|tw|th
        dn = bufA[:, 72:80]
        whr = WH.rearrange("p (a b) -> p a b", b=W)
        w_in = whr[:, 0::2, :]  # pw|tw  [128,2,9]
        h_in = whr[:, 1::2, :]  # ph|th  [128,2,9]

        def r2(ap):             # view a [128,18] tile as [128,2,9]
            return ap.rearrange("p (a b) -> p a b", b=W)

        # === Pool preamble (no data deps: runs during the input DMA) ===
        stats = T(3, "stats")
        ones8 = T(FREE, "ones8")
        b25 = T(1, "b25")
        beps = T(1, "beps")
        G.memset(stats[:, 2:3], 0.0)
        G.memset(ones8[:], 1.0)
        G.memset(b25[:], 2.5)
        G.memset(beps[:], 1e-12)

        # ACT: d2 = sum((ei-ej)^2) fused square+row-reduce (diff itself is
        # emitted later in the Pool queue so it can't stall the arctan chain)
        diff = T(D, "diff")
        sq2 = T(D, "sq2")
        d2c = T(1, "d2c")
        S.activation(sq2[:], diff[:], AF.Square, accum_out=d2c[:])
        lnd2 = T(1, "lnd2")
        S.activation(lnd2[:], d2c[:], AF.Ln, bias=beps[:])
        dist = T(1, "dist")
        S.activation(dist[:], lnd2[:], AF.Exp, scale=0.5)
        rlu = T(1, "rlu")       # relu(1 - dist)
        S.activation(rlu[:], dist[:], AF.Relu, scale=-1.0, bias=1.0)

        # === Pool: box-side prep ===
        za = T(18, "za")        # w - h (pred | targ)
        zb = T(18, "zb")        # w + h
        G.tensor_tensor(r2(za[:]), w_in, h_in, A.subtract)
        G.tensor_tensor(r2(zb[:]), w_in, h_in, A.add)
        dwt = T(FREE, "dwt")    # 1 + 1.2*density
        G.tensor_scalar(dwt[:], dn, ALPHA_D, 1.0, A.mult, A.add)
        ar = T(18, "ar")        # a1|a2 = pw*ph | tw*th
        G.tensor_tensor(r2(ar[:]), w_in, h_in, A.mult)
        u0 = T(W, "u0")
        G.tensor_tensor(u0[:], ar[:, 0:W], ar[:, W:2 * W], A.add)
        u0e = T(W, "u0e")       # a1 + a2 + EPS
        G.tensor_scalar(u0e[:], u0[:], EPS, None, A.add)
        ad = T(FREE, "ad")      # area + 1e-7 (shard cols of tw*th)
        G.tensor_scalar(ad[:], ar[:, W:W + FREE], 1e-7, None, A.add)

        # === DVE: arctan ratio + corners chain ===
        rzb = T(18, "rzb")
        V.reciprocal(rzb[:], zb[:])
        z = T(18, "z")
        V.tensor_tensor(z[:], za[:], rzb[:], A.mult)
        uu = T(18, "uu")        # z^2
        V.tensor_tensor(uu[:], z[:], z[:], A.mult)
        lo = T(36, "lo")        # b1x1|b1y1|b2x1|b2y1
        hi = T(36, "hi")        # b1x2|b1y2|b2x2|b2y2
        V.scalar_tensor_tensor(lo[:], WH, -0.5, P2, A.mult, A.add)
        V.scalar_tensor_tensor(hi[:], WH, 0.5, P2, A.mult, A.add)
        mlo = T(18, "mlo")
        mhi = T(18, "mhi")
        V.tensor_tensor(mlo[:], lo[:, 0:18], lo[:, 18:36], A.max)
        V.tensor_tensor(mhi[:], hi[:, 0:18], hi[:, 18:36], A.min)
        iwh = T(18, "iwh")
        V.tensor_tensor(iwh[:], mhi[:], mlo[:], A.subtract)
        iwr = T(18, "iwr")
        V.tensor_scalar_max(iwr[:], iwh[:], 0.0)
        inter = T(W, "inter")
        V.tensor_tensor(inter[:], iwr[:, 0:W], iwr[:, W:2 * W], A.mult)
        u2 = T(W, "u2")         # union = u0e - inter
        V.scalar_tensor_tensor(u2[:], inter[:], -1.0, u0e[:], A.mult, A.add)
        ru = T(W, "ru")
        V.reciprocal(ru[:], u2[:])
        iou = T(W, "iou")
        V.tensor_tensor(iou[:], inter[:], ru[:], A.mult)
        w1p = T(W, "w1p")       # (1+EPS) - iou
        V.tensor_scalar(w1p[:], iou[:], -1.0, 1.0 + EPS, A.mult, A.add)

        # === Pool: arctan poly head (needs uu from DVE) ===
        h1 = T(18, "h1")        # c3 + c5*u
        G.tensor_scalar(h1[:], uu[:], AT_C5, AT_C3, A.mult, A.add)
        hu = T(18, "hu")
        G.tensor_tensor(hu[:], h1[:], uu[:], A.mult)
        # DVE: ats = (hu + c1) * z   (STT only exists on DVE)
        ats = T(18, "ats")
        V.scalar_tensor_tensor(ats[:], hu[:], AT_C1, z[:], A.add, A.mult)
        # Pool: v pieces
        dv = T(W, "dv")
        G.tensor_tensor(dv[:], ats[:, W:2 * W], ats[:, 0:W], A.subtract)
        dv2 = T(W, "dv2")
        G.tensor_tensor(dv2[:], dv[:], dv[:], A.mult)
        vv = T(W, "vv")         # dv2^2
        G.tensor_tensor(vv[:], dv2[:], dv2[:], A.mult)
        G.tensor_tensor(diff[:], bufB[:, 0:D], bufB[:, D:2 * D], A.subtract)

        # === DVE: rho2 / enclosing-box chains (fill the dv2 wait) ===
        c0 = T(18, "c0")
        c1 = T(18, "c1")
        V.tensor_tensor(c0[:], hi[:, 0:18], hi[:, 18:36], A.max)
        V.tensor_tensor(c1[:], lo[:, 0:18], lo[:, 18:36], A.min)
        dxy = T(18, "dxy")
        V.tensor_tensor(dxy[:], P2[:, 18:36], P2[:, 0:18], A.subtract)
        dsq = T(18, "dsq")
        V.tensor_tensor(dsq[:], dxy[:], dxy[:], A.mult)
        rho2 = T(W, "rho2")
        V.tensor_tensor(rho2[:], dsq[:, 0:W], dsq[:, W:2 * W], A.add)
        cwh = T(18, "cwh")
        V.tensor_tensor(cwh[:], c0[:], c1[:], A.subtract)
        csq = T(18, "csq")
        V.tensor_tensor(csq[:], cwh[:], cwh[:], A.mult)
        c2e = T(W, "c2e")       # cw^2 + ch^2 + EPS
        V.scalar_tensor_tensor(c2e[:], csq[:, 0:W], EPS, csq[:, W:2 * W],
                               A.add, A.add)
        rc2 = T(W, "rc2")
        V.reciprocal(rc2[:], c2e[:])
        rr = T(W, "rr")         # rho2 / c2
        V.tensor_tensor(rr[:], rho2[:], rc2[:], A.mult)
        omirr = T(W, "omirr")   # (1-iou) + rho2/c2   (+EPS, negligible)
        V.tensor_tensor(omirr[:], w1p[:], rr[:], A.add)
        d1 = T(W, "d1")         # v + (1+EPS) - iou
        V.scalar_tensor_tensor(d1[:], dv2[:], VS, w1p[:], A.mult, A.add)
        rd = T(W, "rd")
        V.reciprocal(rd[:], d1[:])
        va = T(W, "va")         # v^2 / d1 = (VS^2*vv) * rd = v*alpha
        V.scalar_tensor_tensor(va[:], vv[:], VS * VS, rd[:],
                               A.mult, A.mult)
        om = T(W, "om")         # 1 - ciou
        V.tensor_tensor(om[:], omirr[:], va[:], A.add)

        # === Pool: mask (pair col of iou) + h2 ===
        mask = pool.tile([PPART, 1], mybir.dt.int32, name="mask", tag="mask")
        G.tensor_scalar(mask[:], iou[:, FREE:W], TAU, None, A.is_gt)
        h2 = T(1, "h2")
        G.tensor_tensor(h2[:], rlu[:], rlu[:], A.mult)

        # === ACT tail: e5 | ln(om) | p25 ===
        e5 = T(FREE, "e5")      # exp(-5*om + 2.5)
        S.activation(e5[:], om[:, 0:FREE], AF.Exp, scale=-5.0, bias=b25[:])
        lnom = T(FREE, "lnom")
        S.activation(lnom[:], om[:, 0:FREE], AF.Ln)
        p25 = T(FREE, "p25")    # om^2.5
        S.activation(p25[:], lnom[:], AF.Exp, scale=GAMMA)

        # === DVE tail ===
        # b-partial: 1/ad with fused row-reduce into stats col 1
        ia = T(FREE, "ia")
        V.reciprocal(ia[:], ad[:])
        ib = T(FREE, "ib")
        V.tensor_scalar(ib[:], ia[:], 1.0, 0.0, A.mult, A.add,
                        accum_out=stats[:, 1:2])
        t1 = T(FREE, "t1")      # 1 + e5
        V.tensor_scalar_add(t1[:], e5[:], 1.0)
        rt = T(FREE, "rt")
        V.reciprocal(rt[:], t1[:])
        m1 = T(FREE, "m1")      # dw * hw = dwt * rt
        V.tensor_tensor(m1[:], dwt[:], rt[:], A.mult)
        scr = T(FREE, "scr")    # m1 * p25, row-reduced into stats col 0
        V.scalar_tensor_tensor(scr[:], m1[:], 1.0, p25[:], A.mult, A.mult,
                               accum_out=stats[:, 0:1])
        V.copy_predicated(stats[:, 2:3], mask[:], h2[:])

        # === PE: partition reduce -> [1,3]; single-packet DMA out ===
        pt = ppool.tile([PPART, 3], dt, name="pt", tag="pt")
        nc.tensor.matmul(pt[0:1, :], ones8[:, 0:1], stats[:],
                         start=True, stop=True)
        outs = pool.tile([1, 3], dt, name="outs", tag="outs")
        S.activation(outs[:], pt[0:1, :], AF.Copy)
        nc.sync.dma_start(out_d.ap(), outs[:])

    nc.compile()
    return nc


def _get_nc():
    global _BUILT
    if _BUILT is None:
        _BUILT = _build_nc()
    return _BUILT


def _pack_inputs(pred_boxes, target_boxes, embeddings, density_map, indices):
    pred = np.ascontiguousarray(pred_boxes, dtype=np.float32)
    targ = np.ascontiguousarray(target_boxes, dtype=np.float32)
    emb = np.ascontiguousarray(embeddings, dtype=np.float32)
    dens = np.ascontiguousarray(density_map, dtype=np.float32)
    idx = np.asarray(indices).astype(np.int64)

    i0, i1 = idx[:, 0], idx[:, 1]
    # dummy pad pairs: far-apart unit boxes -> iou 0 -> mask 0
    bi_all = np.tile(np.array([1.0, 1.0, 1.0, 1.0], np.float32),
                     (N_CORES * PAIRS_PER, 1))
    bj_all = np.tile(np.array([9.0, 9.0, 1.0, 1.0], np.float32),
                     (N_CORES * PAIRS_PER, 1))
    ei_all = np.zeros((N_CORES * PAIRS_PER, D), np.float32)
    ej_all = np.zeros((N_CORES * PAIRS_PER, D), np.float32)
    bi_all[:NPAIR] = pred[i0]
    bj_all[:NPAIR] = pred[i1]
    ei_all[:NPAIR] = emb[i0]
    ej_all[:NPAIR] = emb[i1]

    in_maps = []
    for c in range(N_CORES):
        s = slice(c * NS, (c + 1) * NS)
        pbs = pred[s].reshape(PPART, FREE, 4)
        tbs = targ[s].reshape(PPART, FREE, 4)
        ps = slice(c * PAIRS_PER, (c + 1) * PAIRS_PER)
        bi, bj = bi_all[ps], bj_all[ps]
        bufA = np.empty((PPART, 80), np.float32)
        # P2 blocks: px py tx ty ; WH blocks: pw ph tw th
        for k, (src, comp) in enumerate(
                [(pbs, 0), (pbs, 1), (tbs, 0), (tbs, 1),
                 (pbs, 2), (pbs, 3), (tbs, 2), (tbs, 3)]):
            pair = (bi if src is pbs else bj)[:, comp]
            bufA[:, k * W:k * W + FREE] = src[:, :, comp]
            bufA[:PAIRS_PER, k * W + FREE] = pair
            bufA[PAIRS_PER:, k * W + FREE] = 1.0 if src is pbs else 9.0
            if comp < 2 and src is not pbs:
                pass
        # fix pad rows of pair col: w/h must be 1.0 for both
        for k, (src, comp) in enumerate(
                [(pbs, 0), (pbs, 1), (tbs, 0), (tbs, 1),
                 (pbs, 2), (pbs, 3), (tbs, 2), (tbs, 3)]):
            if comp >= 2:
                bufA[PAIRS_PER:, k * W + FREE] = 1.0
        bufA[:, 72:80] = dens[s].reshape(PPART, FREE)
        bufB = np.concatenate([ei_all[ps], ej_all[ps]], axis=1)
        in_maps.append({"bufA": bufA, "bufB": np.ascontiguousarray(bufB)})
    return in_maps


def kernel(pred_boxes, target_boxes, embeddings, density_map, indices):
    global LAST_RESULT
    import time as _time

    from concourse.bass_utils import run_bass_kernel_spmd

    nc = _get_nc()
    in_maps = _pack_inputs(pred_boxes, target_boxes, embeddings,
                           density_map, indices)
    for attempt in range(3):
        try:
            res = run_bass_kernel_spmd(nc, in_maps,
                                       core_ids=list(range(N_CORES)))
            break
        except Exception:
            # a crashed earlier run can leave a core wedged
            # (NRT_EXEC_UNIT_UNRECOVERABLE); it clears on retry
            if attempt == 2:
                raise
            _time.sleep(2.0)
    LAST_RESULT = res

    outs = np.stack([res.results[c]["out"] for c in range(N_CORES)])  # [8,1,3]
    s_a = float(np.sum(outs[:, 0, 0], dtype=np.float64))
    s_b = float(np.sum(outs[:, 0, 1], dtype=np.float64))
    contrast = float(np.sum(outs[:, 0, 2], dtype=np.float64))
    loss = s_a * s_b / (N * N) + LAMBDA_C * contrast / (NPAIR + 1e-7)
    return np.asarray(np.float32(loss))


# revision 15
# speedup vs baseline: 1.2359x; 1.0050x over previous
"""DOSACon loss on 8 Trainium2 NeuronCores (Bass/Tile, SPMD data-parallel).

Math: the [N,N] broadcast in the localization term is rank-1 separable --
  mean(dw * hw * (1-ciou)^g / (area+eps)) over [N,N]
    = (sum_i dw_i*hw_i*(1-ciou_i)^g) * (sum_j 1/(area_j+eps)) / N^2
so each core computes partial sums over its 1024-row shard of the N=8192
boxes.  The 100 contrastive pairs are gathered on host (pure data
movement) and SHARDED across cores (13 per core) in a packed pair lane.

Single activation table (#6: ln/exp):
  sigmoid(5(0.5-ciou)) = 1/(1+exp(-5*om+2.5))   with om = 1-ciou
  (1-ciou)^2.5        = exp(2.5*ln(om))
  ||ei-ej||           = exp(0.5*ln(d2+1e-12))
  arctan              = odd deg-5 polynomial of z=(w-h)/(w+h) (1.4e-5
                        end-to-end error), so no Arctan table needed.

Per-core inputs: bufA [128, 80] f32 (box data: P2|WH|density, 9-wide
blocks = 8 shard cols + 1 pair col), bufB [13, 512] (13 pairs ei|ej).
Output: [1, 3] = (sum_a, sum_b, sum_pair) after an on-device partition
reduction via PE matmul against a ones column -- a single 12B DMA packet.

Engine plan: DVE owns the serial CIoU chain; Pool does the arctan poly,
area/density prep, embedding diff, mask; ACT does the ln/exp ops plus the
squared-distance accumulation (Square with accum_out); PE does the final
partition reduce.  Fused reduce: scr/ib use accum_out to fold the X-axis
reduction into the last elementwise op.
"""

from contextlib import ExitStack

import numpy as np

N_CORES = 8
N = 8192
NS = N // N_CORES      # 1024 boxes per core
PPART = 128            # SBUF partitions
FREE = NS // PPART     # 8 shard columns
W = FREE + 1           # 9 = shard columns + 1 pair column
D = 256
NPAIR = 100
PAIRS_PER = 13         # ceil(100/8); tail cores padded with dummies

GAMMA = 2.5
ALPHA_D = 1.2
DELTA = 1.0
TAU = 0.3
LAMBDA_C = 0.5
EPS = 1e-7
VS = 4.0 / np.pi ** 2
# odd minimax-ish arctan poly on [-1,1]: c1*z + c3*z^3 + c5*z^5
AT_C1 = 0.99570612
AT_C3 = -0.29065729
AT_C5 = 0.08132208

_BUILT = None          # cached nc across calls
LAST_RESULT = None     # last BassKernelResults (for profiling in test.py)


def _build_nc():
    import concourse.bacc as bacc
    import concourse.mybir as mybir
    import concourse.tile as tile
    from concourse.tile import add_dep_helper

    dt = mybir.dt.float32
    A = mybir.AluOpType
    AF = mybir.ActivationFunctionType

    nc = bacc.Bacc("TRN2", target_bir_lowering=False, debug=False,
                   num_devices=N_CORES, enable_partition_id=False)
    bufA_d = nc.dram_tensor("bufA", [PPART, 80], dt, kind="ExternalInput")
    bufB_d = nc.dram_tensor("bufB", [PAIRS_PER, 2 * D], dt,
                            kind="ExternalInput")
    out_d = nc.dram_tensor("out", [1, 3], dt, kind="ExternalOutput")

    with tile.TileContext(nc) as tc, ExitStack() as ctx:
        pool = ctx.enter_context(tc.tile_pool(name="p", bufs=1))
        ppool = ctx.enter_context(tc.psum_pool(name="pp", bufs=1))

        def T(n, tag, p=PPART):
            return pool.tile([p, n], dt, name=tag, tag=tag)

        V, S, G = nc.vector, nc.scalar, nc.gpsimd

        # load act table 6 (ln/exp/square/relu/copy) once, up front; the
        # compiler's per-function greedy table choice would thrash 0<->5.
        tl = mybir.InstLoadActFuncSet(
            name=nc.get_next_instruction_name(), ins=[], outs=[])
        tl.act_func_set_id = 6
        S.add_instruction(tl)

        bufA = T(80, "bufA")
        bufB = T(2 * D, "bufB")
        nc.sync.dma_start(bufA[:], bufA_d.ap())
        nc.sync.dma_start(bufB[0:PAIRS_PER, :], bufB_d.ap())

        P2 = bufA[:, 0:36]      # px|py|tx|ty (9-wide blocks)
        WH = bufA[:, 36:72]     # pw|ph|tw|th
        dn = bufA[:, 72:80]
        whr = WH.rearrange("p (a b) -> p a b", b=W)
        w_in = whr[:, 0::2, :]  # pw|tw  [128,2,9]
        h_in = whr[:, 1::2, :]  # ph|th  [128,2,9]

        def r2(ap):             # view a [128,18] tile as [128,2,9]
            return ap.rearrange("p (a b) -> p a b", b=W)

        # === Pool preamble (no data deps: runs during the input DMA) ===
        stats = T(3, "stats")
        ones8 = T(FREE, "ones8")
        b25 = T(1, "b25")
        beps = T(1, "beps")
        G.memset(stats[:, 2:3], 0.0)
        G.memset(ones8[:], 1.0)
        G.memset(b25[:], 2.5)
        G.memset(beps[:], 1e-12)

        # === Pool: box-side prep ===
        za = T(18, "za")        # w - h (pred | targ)
        zb = T(18, "zb")        # w + h
        G.tensor_tensor(r2(za[:]), w_in, h_in, A.subtract)
        G.tensor_tensor(r2(zb[:]), w_in, h_in, A.add)
        dwt = T(FREE, "dwt")    # 1 + 1.2*density
        G.tensor_scalar(dwt[:], dn, ALPHA_D, 1.0, A.mult, A.add)
        ar = T(18, "ar")        # a1|a2 = pw*ph | tw*th
        G.tensor_tensor(r2(ar[:]), w_in, h_in, A.mult)
        u0 = T(W, "u0")
        G.tensor_tensor(u0[:], ar[:, 0:W], ar[:, W:2 * W], A.add)
        u0e = T(W, "u0e")       # a1 + a2 + EPS
        G.tensor_scalar(u0e[:], u0[:], EPS, None, A.add)
        ad = T(FREE, "ad")      # area + 1e-7 (shard cols of tw*th)
        G.tensor_scalar(ad[:], ar[:, W:W + FREE], 1e-7, None, A.add)

        # === DVE: arctan ratio + corners chain ===
        rzb = T(18, "rzb")
        V.reciprocal(rzb[:], zb[:])
        z = T(18, "z")
        V.tensor_tensor(z[:], za[:], rzb[:], A.mult)
        uu = T(18, "uu")        # z^2
        V.tensor_tensor(uu[:], z[:], z[:], A.mult)
        lo = T(36, "lo")        # b1x1|b1y1|b2x1|b2y1
        hi = T(36, "hi")        # b1x2|b1y2|b2x2|b2y2
        V.scalar_tensor_tensor(lo[:], WH, -0.5, P2, A.mult, A.add)
        V.scalar_tensor_tensor(hi[:], WH, 0.5, P2, A.mult, A.add)
        mlo = T(18, "mlo")
        mhi = T(18, "mhi")
        V.tensor_tensor(mlo[:], lo[:, 0:18], lo[:, 18:36], A.max)
        V.tensor_tensor(mhi[:], hi[:, 0:18], hi[:, 18:36], A.min)
        iwh = T(18, "iwh")
        V.tensor_tensor(iwh[:], mhi[:], mlo[:], A.subtract)
        iwr = T(18, "iwr")
        V.tensor_scalar_max(iwr[:], iwh[:], 0.0)
        inter = T(W, "inter")
        V.tensor_tensor(inter[:], iwr[:, 0:W], iwr[:, W:2 * W], A.mult)
        u2 = T(W, "u2")         # union = u0e - inter
        V.scalar_tensor_tensor(u2[:], inter[:], -1.0, u0e[:], A.mult, A.add)
        ru = T(W, "ru")
        V.reciprocal(ru[:], u2[:])
        iou = T(W, "iou")
        V.tensor_tensor(iou[:], inter[:], ru[:], A.mult)
        w1p = T(W, "w1p")       # (1+EPS) - iou
        V.tensor_scalar(w1p[:], iou[:], -1.0, 1.0 + EPS, A.mult, A.add)

        # === Pool: arctan poly head (needs uu from DVE) ===
        h1 = T(18, "h1")        # c3 + c5*u
        G.tensor_scalar(h1[:], uu[:], AT_C5, AT_C3, A.mult, A.add)
        hu = T(18, "hu")
        G.tensor_tensor(hu[:], h1[:], uu[:], A.mult)
        # DVE: ats = (hu + c1) * z   (STT only exists on DVE)
        ats = T(18, "ats")
        V.scalar_tensor_tensor(ats[:], hu[:], AT_C1, z[:], A.add, A.mult)
        # Pool: v pieces
        dv = T(W, "dv")
        G.tensor_tensor(dv[:], ats[:, W:2 * W], ats[:, 0:W], A.subtract)
        dv2 = T(W, "dv2")
        G.tensor_tensor(dv2[:], dv[:], dv[:], A.mult)
        vv = T(W, "vv")         # dv2^2
        vv_i = G.tensor_tensor(vv[:], dv2[:], dv2[:], A.mult)
        # embedding diff strictly after the v-chain on Pool (the scheduler
        # otherwise hoists this 256-wide op in front of the critical chain)
        diff = T(D, "diff")
        diff_i = G.tensor_tensor(diff[:], bufB[:, 0:D], bufB[:, D:2 * D],
                                 A.subtract)
        add_dep_helper(diff_i.ins, vv_i.ins, sync=False,
                       reason="order Pool diff after vv")
        # ACT: d2 = sum((ei-ej)^2) fused square+row-reduce, then hinge
        sq2 = T(D, "sq2")
        d2c = T(1, "d2c")
        S.activation(sq2[:], diff[:], AF.Square, accum_out=d2c[:])
        lnd2 = T(1, "lnd2")
        S.activation(lnd2[:], d2c[:], AF.Ln, bias=beps[:])
        dist = T(1, "dist")
        S.activation(dist[:], lnd2[:], AF.Exp, scale=0.5)
        rlu = T(1, "rlu")       # relu(1 - dist)
        S.activation(rlu[:], dist[:], AF.Relu, scale=-1.0, bias=1.0)

        # === DVE: rho2 / enclosing-box chains (fill the dv2 wait) ===
        c0 = T(18, "c0")
        c1 = T(18, "c1")
        V.tensor_tensor(c0[:], hi[:, 0:18], hi[:, 18:36], A.max)
        V.tensor_tensor(c1[:], lo[:, 0:18], lo[:, 18:36], A.min)
        dxy = T(18, "dxy")
        V.tensor_tensor(dxy[:], P2[:, 18:36], P2[:, 0:18], A.subtract)
        dsq = T(18, "dsq")
        V.tensor_tensor(dsq[:], dxy[:], dxy[:], A.mult)
        rho2 = T(W, "rho2")
        V.tensor_tensor(rho2[:], dsq[:, 0:W], dsq[:, W:2 * W], A.add)
        cwh = T(18, "cwh")
        V.tensor_tensor(cwh[:], c0[:], c1[:], A.subtract)
        csq = T(18, "csq")
        V.tensor_tensor(csq[:], cwh[:], cwh[:], A.mult)
        c2e = T(W, "c2e")       # cw^2 + ch^2 + EPS
        V.scalar_tensor_tensor(c2e[:], csq[:, 0:W], EPS, csq[:, W:2 * W],
                               A.add, A.add)
        rc2 = T(W, "rc2")
        V.reciprocal(rc2[:], c2e[:])
        rr = T(W, "rr")         # rho2 / c2
        V.tensor_tensor(rr[:], rho2[:], rc2[:], A.mult)
        omirr = T(W, "omirr")   # (1-iou) + rho2/c2   (+EPS, negligible)
        V.tensor_tensor(omirr[:], w1p[:], rr[:], A.add)
        d1 = T(W, "d1")         # v + (1+EPS) - iou
        V.scalar_tensor_tensor(d1[:], dv2[:], VS, w1p[:], A.mult, A.add)
        rd = T(W, "rd")
        V.reciprocal(rd[:], d1[:])
        va = T(W, "va")         # v^2 / d1 = (VS^2*vv) * rd = v*alpha
        V.scalar_tensor_tensor(va[:], vv[:], VS * VS, rd[:],
                               A.mult, A.mult)
        om = T(W, "om")         # 1 - ciou
        V.tensor_tensor(om[:], omirr[:], va[:], A.add)

        # === Pool: mask (pair col of iou) + h2 ===
        mask = pool.tile([PPART, 1], mybir.dt.int32, name="mask", tag="mask")
        G.tensor_scalar(mask[:], iou[:, FREE:W], TAU, None, A.is_gt)
        h2 = T(1, "h2")
        G.tensor_tensor(h2[:], rlu[:], rlu[:], A.mult)

        # === ACT tail: e5 | ln(om) | p25 ===
        e5 = T(FREE, "e5")      # exp(-5*om + 2.5)
        S.activation(e5[:], om[:, 0:FREE], AF.Exp, scale=-5.0, bias=b25[:])
        lnom = T(FREE, "lnom")
        S.activation(lnom[:], om[:, 0:FREE], AF.Ln)
        p25 = T(FREE, "p25")    # om^2.5
        S.activation(p25[:], lnom[:], AF.Exp, scale=GAMMA)

        # === DVE tail ===
        # b-partial: 1/ad with fused row-reduce into stats col 1
        ia = T(FREE, "ia")
        V.reciprocal(ia[:], ad[:])
        ib = T(FREE, "ib")
        V.tensor_scalar(ib[:], ia[:], 1.0, 0.0, A.mult, A.add,
                        accum_out=stats[:, 1:2])
        t1 = T(FREE, "t1")      # 1 + e5
        V.tensor_scalar_add(t1[:], e5[:], 1.0)
        rt = T(FREE, "rt")
        V.reciprocal(rt[:], t1[:])
        m1 = T(FREE, "m1")      # dw * hw = dwt * rt
        V.tensor_tensor(m1[:], dwt[:], rt[:], A.mult)
        scr = T(FREE, "scr")    # m1 * p25, row-reduced into stats col 0
        V.scalar_tensor_tensor(scr[:], m1[:], 1.0, p25[:], A.mult, A.mult,
                               accum_out=stats[:, 0:1])
        V.copy_predicated(stats[:, 2:3], mask[:], h2[:])

        # === PE: partition reduce -> [1,3]; single-packet DMA out ===
        pt = ppool.tile([PPART, 3], dt, name="pt", tag="pt")
        nc.tensor.matmul(pt[0:1, :], ones8[:, 0:1], stats[:],
                         start=True, stop=True)
        outs = pool.tile([1, 3], dt, name="outs", tag="outs")
        S.activation(outs[:], pt[0:1, :], AF.Copy)
        nc.sync.dma_start(out_d.ap(), outs[:])

    nc.compile()
    return nc


def _get_nc():
    global _BUILT
    if _BUILT is None:
        _BUILT = _build_nc()
    return _BUILT


def _pack_inputs(pred_boxes, target_boxes, embeddings, density_map, indices):
    pred = np.ascontiguousarray(pred_boxes, dtype=np.float32)
    targ = np.ascontiguousarray(target_boxes, dtype=np.float32)
    emb = np.ascontiguousarray(embeddings, dtype=np.float32)
    dens = np.ascontiguousarray(density_map, dtype=np.float32)
    idx = np.asarray(indices).astype(np.int64)

    i0, i1 = idx[:, 0], idx[:, 1]
    # dummy pad pairs: far-apart unit boxes -> iou 0 -> mask 0
    bi_all = np.tile(np.array([1.0, 1.0, 1.0, 1.0], np.float32),
                     (N_CORES * PAIRS_PER, 1))
    bj_all = np.tile(np.array([9.0, 9.0, 1.0, 1.0], np.float32),
                     (N_CORES * PAIRS_PER, 1))
    ei_all = np.zeros((N_CORES * PAIRS_PER, D), np.float32)
    ej_all = np.zeros((N_CORES * PAIRS_PER, D), np.float32)
    bi_all[:NPAIR] = pred[i0]
    bj_all[:NPAIR] = pred[i1]
    ei_all[:NPAIR] = emb[i0]
    ej_all[:NPAIR] = emb[i1]

    in_maps = []
    for c in range(N_CORES):
        s = slice(c * NS, (c + 1) * NS)
        pbs = pred[s].reshape(PPART, FREE, 4)
        tbs = targ[s].reshape(PPART, FREE, 4)
        ps = slice(c * PAIRS_PER, (c + 1) * PAIRS_PER)
        bi, bj = bi_all[ps], bj_all[ps]
        bufA = np.empty((PPART, 80), np.float32)
        # P2 blocks: px py tx ty ; WH blocks: pw ph tw th
        for k, (src, comp) in enumerate(
                [(pbs, 0), (pbs, 1), (tbs, 0), (tbs, 1),
                 (pbs, 2), (pbs, 3), (tbs, 2), (tbs, 3)]):
            pair = (bi if src is pbs else bj)[:, comp]
            bufA[:, k * W:k * W + FREE] = src[:, :, comp]
            bufA[:PAIRS_PER, k * W + FREE] = pair
            bufA[PAIRS_PER:, k * W + FREE] = 1.0 if src is pbs else 9.0
            if comp < 2 and src is not pbs:
                pass
        # fix pad rows of pair col: w/h must be 1.0 for both
        for k, (src, comp) in enumerate(
                [(pbs, 0), (pbs, 1), (tbs, 0), (tbs, 1),
                 (pbs, 2), (pbs, 3), (tbs, 2), (tbs, 3)]):
            if comp >= 2:
                bufA[PAIRS_PER:, k * W + FREE] = 1.0
        bufA[:, 72:80] = dens[s].reshape(PPART, FREE)
        bufB = np.concatenate([ei_all[ps], ej_all[ps]], axis=1)
        in_maps.append({"bufA": bufA, "bufB": np.ascontiguousarray(bufB)})
    return in_maps


def kernel(pred_boxes, target_boxes, embeddings, density_map, indices):
    global LAST_RESULT
    import time as _time

    from concourse.bass_utils import run_bass_kernel_spmd

    nc = _get_nc()
    in_maps = _pack_inputs(pred_boxes, target_boxes, embeddings,
                           density_map, indices)
    for attempt in range(3):
        try:
            res = run_bass_kernel_spmd(nc, in_maps,
                                       core_ids=list(range(N_CORES)))
            break
        except Exception:
            # a crashed earlier run can leave a core wedged
            # (NRT_EXEC_UNIT_UNRECOVERABLE); it clears on retry
            if attempt == 2:
                raise
            _time.sleep(2.0)
    LAST_RESULT = res

    outs = np.stack([res.results[c]["out"] for c in range(N_CORES)])  # [8,1,3]
    s_a = float(np.sum(outs[:, 0, 0], dtype=np.float64))
    s_b = float(np.sum(outs[:, 0, 1], dtype=np.float64))
    contrast = float(np.sum(outs[:, 0, 2], dtype=np.float64))
    loss = s_a * s_b / (N * N) + LAMBDA_C * contrast / (NPAIR + 1e-7)
    return np.asarray(np.float32(loss))
